# revision 1
# baseline (speedup 1.0000x reference)
"""Trainium2 Bass kernel for nn_EntityResolution (segment_reduce).

Strategy (8 cores, single launch, one AllReduce):
  - The 307MB embedding table is row-sharded: core k holds rows
    [k*12500, (k+1)*12500) of emb_weight, pre-transposed on host to
    wt = W.T shard [768, 12500].
  - Phase 1: V_k[t, c] = sum_e wt[e, t] * spansT[e, c] for all 128
    columns c = b*16 + s  (fp32r matmul, PE-transposed, written to DRAM
    as [12500, 128] rows).
  - Phase 2: every element (any batch) whose triplet id falls in shard k
    is processed on core k. Host assigns each element a slot
    (p = b*16 + m%16, i) and ships: gidx (int16 local row ids, wrapped
    for dma_gather) and satt (att value one-hot over j2 = m//16).
    dma_gather pulls 512B V rows; an identity-mask multiply+reduce
    extracts the diagonal V[lid, p]; a second multiply+reduce against
    satt yields partial sum1 [128 (b,s), 32 (j2)].
  - Phase 3: 16KB AllReduce combines the 8 partial sums.
  - Phase 4: softmax over s' (= m//32, a strided free-axis view),
    span-score multiply (hosted-mask matmul), own-batch extraction
    (hosted one-hot matmul), 512-softmax, duplicate-entity resolution
    (is_equal compare matrix), and the 1M-entity softmax emitted as a
    constant fill plus 512 scattered values.
"""
import os
import sys
sys.path.insert(0, '/opt/trn_rl_repo')

import numpy as np

import concourse.bass as bass
import concourse.bacc as bacc
import concourse.mybir as mybir
import concourse.tile as tile
from concourse import library_config
from concourse.masks import make_identity
from concourse.bass_utils import run_bass_kernel_spmd

# problem shapes (hardcoded; kernel.py must be self-contained)
B, S, C, PB, E = 8, 16, 32, 64, 768
M = S * C                # 512 bags per batch
L = M * PB               # 32768 triplet ids per batch
T = 100000               # triplet vocab
NE = 1000000             # entities
N_CORES = 8
TS = T // N_CORES        # 12500 shard rows
HALF = 6500              # v table split (multiple of 500)
NCH = 16                 # gather chunks
OUT_W = 7824             # out [128, 7824] -> flat 1001472 >= NE+1
FILL_W = OUT_W // 4

AX = mybir.AxisListType
OP = mybir.AluOpType
ACT = mybir.ActivationFunctionType
dt = mybir.dt

_cache = {}


def _build(nslot):
    phase = float(os.environ.get("K_PHASE", "9"))
    chi = nslot // NCH
    nidx = chi * 128                 # idxs per dma_gather chunk
    gw = nidx // 16                  # wrapped idx width per chunk
    nc = bacc.Bacc("TRN2", target_bir_lowering=False, debug=False,
                   num_devices=N_CORES)

    wt = nc.dram_tensor("wt", [E, TS], dt.float32, kind="ExternalInput")
    spansT = nc.dram_tensor("spansT", [E, 128], dt.float32, kind="ExternalInput")
    spans_all = nc.dram_tensor("spans_all", [128, E], dt.float32, kind="ExternalInput")
    spanw = nc.dram_tensor("spanw", [128, E], dt.float32, kind="ExternalInput")
    spanb = nc.dram_tensor("spanb", [128, 1], dt.float32, kind="ExternalInput")
    gidx = nc.dram_tensor("gidx", [128, NCH * gw], dt.int16, kind="ExternalInput")
    satt = nc.dram_tensor("satt", [128, NCH * 32 * chi], dt.float32,
                          kind="ExternalInput")
    hostb = nc.dram_tensor("hostb", [128, 128], dt.float32, kind="ExternalInput")
    hostm = nc.dram_tensor("hostm", [128, 32], dt.float32, kind="ExternalInput")
    hostown = nc.dram_tensor("hostown", [128, 16], dt.float32, kind="ExternalInput")
    qidp_i = nc.dram_tensor("qidp_i", [128, 4], dt.int32, kind="ExternalInput")
    qidp_f = nc.dram_tensor("qidp_f", [128, 4], dt.float32, kind="ExternalInput")
    qidf_free = nc.dram_tensor("qidf_free", [128, 512], dt.float32,
                               kind="ExternalInput")
    out = nc.dram_tensor("out", [128, OUT_W], dt.float32, kind="ExternalOutput")

    rg = [list(range(N_CORES))]

    with tile.TileContext(nc) as tc:
        with (
            tc.tile_pool(name="wtp", bufs=18) as wtp,
            tc.tile_pool(name="vtp", bufs=4) as vtp,
            tc.tile_pool(name="gp", bufs=2) as gp,
            tc.tile_pool(name="t2p", bufs=1) as t2p,
            tc.tile_pool(name="spp", bufs=2) as spp,
            tc.tile_pool(name="sb", bufs=1) as sb,
            tc.tile_pool(name="sm", bufs=1) as sm,
            tc.tile_pool(name="vps", bufs=4, space="PSUM") as vps,
            tc.tile_pool(name="tps", bufs=2, space="PSUM") as tps,
            tc.tile_pool(name="mps", bufs=1, space="PSUM") as mps,
            tc.tile_pool(name="dram", bufs=1, space="DRAM") as dram,
        ):
            nc.gpsimd.load_library(library_config.mlp)
            ident = sb.tile([128, 128], dt.float32)
            make_identity(nc, ident[:])

            # resident small inputs
            spansT_sb = sb.tile([128, 6, 128], dt.float32)
            for e in range(6):
                nc.sync.dma_start(spansT_sb[:, e, :], spansT[e * 128:(e + 1) * 128, :])
            gidx_sb = sb.tile([128, NCH * gw], dt.int16)
            nc.sync.dma_start(gidx_sb[:], gidx[:])

            # ---------- phase 1: V = W @ spans, PE-transposed to [t, c] ----
            # split into two tables so half-A gathers overlap half-B matmuls
            v_a = dram.tile([HALF, 128], dt.float32)
            v_b = dram.tile([TS - HALF, 128], dt.float32)
            # macro-tiles of 1000 t (8 matmul tiles of 125); last macro 500
            macs = [1000] * 12 + [500]
            moff = 0
            for mac in macs:
                wts = []
                for e in range(6):
                    w_t = wtp.tile([128, 1000], dt.float32, tag="wt")
                    nc.sync.dma_start(
                        w_t[:, :mac],
                        wt[e * 128:(e + 1) * 128, moff:moff + mac])
                    wts.append(w_t)
                for ti in range(mac // 125):
                    v_ps = vps.tile([128, 128], dt.float32)
                    for e in range(6):
                        nc.tensor.matmul(
                            v_ps[:125, :],
                            wts[e][:, ti * 125:(ti + 1) * 125],
                            spansT_sb[:, e, :],
                            start=(e == 0), stop=(e == 5))
                    vt = vtp.tile([128, 128], dt.float32, tag="vt")
                    nc.vector.tensor_copy(vt[:125, :], v_ps[:125, :])
                    row = moff + ti * 125
                    vdst = v_a if row < HALF else v_b
                    ro = row if row < HALF else row - HALF
                    nc.scalar.dma_start(vdst[ro: ro + 125, :], vt[:125, :])
                moff += mac

            if phase == 1:
                nc.sync.dma_start(
                    out[:].rearrange("p f -> (p f)")[:128 * 6000, None],
                    v_a[:6000, :].rearrange("a b -> (a b)")[:, None])

            if phase >= 2:
                # ---------- phase 2: gather + diag select + segment matrix -----
                psum1 = sb.tile([128, 32], dt.float32)
                for c in range(NCH):
                    g_t = gp.tile([128, chi, 128], dt.float32, tag="G")
                    nc.gpsimd.dma_gather(
                        out_ap=g_t[:], in_ap=(v_a if c < NCH // 2 else v_b)[:],
                        idxs_ap=gidx_sb[:, c * gw:(c + 1) * gw],
                        num_idxs=nidx, num_idxs_reg=nidx, elem_size=128,
                        single_packet=False)
                    nc.vector.tensor_tensor(
                        out=g_t[:], in0=g_t[:],
                        in1=ident[:, None, :].to_broadcast([128, chi, 128]),
                        op=OP.mult)
                    v1 = spp.tile([128, chi], dt.float32, tag="V1")
                    nc.vector.tensor_reduce(out=v1[:, :, None], in_=g_t[:],
                                            axis=AX.X, op=OP.add)
                    sa = gp.tile([128, 32, chi], dt.float32, tag="SA")
                    nc.scalar.dma_start(
                        sa[:],
                        satt[:, c * 32 * chi:(c + 1) * 32 * chi].rearrange(
                            "p (a b) -> p a b", a=32))
                    t2 = t2p.tile([128, 32, chi], dt.float32, tag="T2")
                    nc.vector.tensor_tensor(
                        out=t2[:], in0=sa[:],
                        in1=v1[:, None, :].to_broadcast([128, 32, chi]),
                        op=OP.mult)
                    psc = spp.tile([128, 32], dt.float32, tag="PSC")
                    nc.vector.tensor_reduce(out=psc[:, :, None], in_=t2[:],
                                            axis=AX.X, op=OP.add)
                    if c == 0:
                        nc.vector.tensor_copy(psum1[:], psc[:])
                    else:
                        nc.vector.tensor_add(psum1[:], psum1[:], psc[:])

                if phase == 2:
                    nc.sync.dma_start(out[:, 0:32], psum1[:])

            # ---------- phase 3: AllReduce [128, 32] -----------------------
            if phase >= 3:
                ar_in = dram.tile([128, 32], dt.float32)
                ar_out = dram.tile([128, 32], dt.float32)
                nc.gpsimd.dma_start(ar_in[:], psum1[:])
                nc.gpsimd.collective_compute(
                    "AllReduce", OP.add, replica_groups=rg,
                    ins=[ar_in.opt()], outs=[ar_out.opt()])
                sum1 = sm.tile([128, 32], dt.float32)
                nc.gpsimd.dma_start(sum1[:], ar_out[:])

                if phase == 3:
                    nc.sync.dma_start(out[:, 0:32], sum1[:])

            # ---------- phase 4: softmaxes ---------------------------------
            if phase >= 3.4:
                # span scores: ssc[p] = span_embs[p] . span_W + b
                spal = sm.tile([128, E], dt.float32)
                spwl = sm.tile([128, E], dt.float32)
                spbl = sm.tile([128, 1], dt.float32)
                nc.sync.dma_start(spal[:], spans_all[:])
                nc.sync.dma_start(spwl[:], spanw[:])
                nc.sync.dma_start(spbl[:], spanb[:])
                tmp768 = sm.tile([128, E], dt.float32)
                nc.vector.tensor_tensor(out=tmp768[:], in0=spal[:], in1=spwl[:],
                                        op=OP.mult)
                ssc = sm.tile([128, 1], dt.float32)
                nc.vector.tensor_reduce(out=ssc[:], in_=tmp768[:], axis=AX.X,
                                        op=OP.add)
                nc.vector.tensor_add(ssc[:], ssc[:], spbl[:])

                # softmax over s' = j2//2 (strided view [128, 2, 16])
                def v216(ap):
                    return ap.rearrange("p (two s2) -> p two s2", two=2)
                mx = sm.tile([128, 2], dt.float32)
                nc.vector.tensor_reduce(out=mx[:, :, None], in_=v216(sum1[:]),
                                        axis=AX.X, op=OP.max)
                e1 = sm.tile([128, 32], dt.float32)
                nc.vector.tensor_tensor(
                    out=v216(e1[:]), in0=v216(sum1[:]),
                    in1=mx[:, :, None].to_broadcast([128, 2, 16]), op=OP.subtract)
                nc.scalar.activation(e1[:], e1[:], ACT.Exp)
                smsum = sm.tile([128, 2], dt.float32)
                nc.vector.tensor_reduce(out=smsum[:, :, None], in_=v216(e1[:]),
                                        axis=AX.X, op=OP.add)
                rsm = sm.tile([128, 2], dt.float32)
                nc.vector.reciprocal(rsm[:], smsum[:])
                nc.vector.tensor_tensor(
                    out=v216(e1[:]), in0=v216(e1[:]),
                    in1=rsm[:, :, None].to_broadcast([128, 2, 16]), op=OP.mult)

                # SSB[p, j2] = span_score(b(p), j2//2) via hosted-mask matmul
                hb = sm.tile([128, 128], dt.float32)
                hm = sm.tile([128, 32], dt.float32)
                ho = sm.tile([128, 16], dt.float32)
                nc.sync.dma_start(hb[:], hostb[:])
                nc.sync.dma_start(hm[:], hostm[:])
                nc.sync.dma_start(ho[:], hostown[:])
                rhsb = sm.tile([128, 32], dt.float32)
                nc.vector.tensor_tensor(out=rhsb[:], in0=hm[:],
                                        in1=ssc[:].to_broadcast([128, 32]),
                                        op=OP.mult)
                ssb_ps = mps.tile([128, 32], dt.float32, tag="mm")
                nc.tensor.matmul(ssb_ps[:], hb[:], rhsb[:], start=True, stop=True)
                mult2 = sm.tile([128, 32], dt.float32)
                nc.vector.tensor_tensor(out=mult2[:], in0=e1[:], in1=ssb_ps[:],
                                        op=OP.mult)

                # own-batch extraction -> [16, 32] -> [1, 512]
                own_ps = mps.tile([16, 32], dt.float32, tag="mm")
                nc.tensor.matmul(own_ps[:], ho[:], mult2[:], start=True, stop=True)
                own = sm.tile([16, 32], dt.float32)
                nc.vector.tensor_copy(own[:], own_ps[:])
                cn = sm.tile([1, 512], dt.float32)
                nc.sync.dma_start(cn[:].rearrange("p (a bb) -> p a bb", a=16), own[:])

                # softmax over 512
                mxn = sm.tile([1, 1], dt.float32)
                nc.vector.tensor_reduce(out=mxn[:], in_=cn[:], axis=AX.X,
                                        op=OP.max, negate=True)
                e5 = sm.tile([1, 512], dt.float32)
                nc.scalar.activation(e5[:], cn[:], ACT.Exp, bias=mxn[:], scale=1.0)
                s5 = sm.tile([1, 1], dt.float32)
                nc.vector.tensor_reduce(out=s5[:], in_=e5[:], axis=AX.X, op=OP.add)
                r5 = sm.tile([1, 1], dt.float32)
                nc.vector.reciprocal(r5[:], s5[:])
                cand = sm.tile([1, 512], dt.float32)
                nc.vector.tensor_tensor(out=cand[:], in0=e5[:],
                                        in1=r5[:].to_broadcast([1, 512]), op=OP.mult)
                if phase == 3.5:
                    nc.sync.dma_start(out[0:1, 0:512], cand[:])


                if phase >= 3.6:
                    # ---------- phase 5: duplicate resolution + output -------------
                    ones128 = sm.tile([1, 128], dt.float32)
                    nc.vector.memset(ones128[:], 1.0)
                    cb_ps = mps.tile([128, 512], dt.float32, tag="mm")
                    nc.tensor.matmul(cb_ps[:], ones128[:], cand[:], start=True, stop=True)
                    candB = sm.tile([128, 512], dt.float32)
                    nc.vector.tensor_copy(candB[:], cb_ps[:])

                    qfp = sm.tile([128, 4], dt.float32)
                    qff = sm.tile([128, 512], dt.float32)
                    qip = sm.tile([128, 4], dt.int32)
                    nc.sync.dma_start(qfp[:], qidp_f[:])
                    nc.sync.dma_start(qff[:], qidf_free[:])
                    nc.sync.dma_start(qip[:], qidp_i[:])

                    eq = sm.tile([128, 4, 512], dt.float32)
                    nc.vector.tensor_tensor(
                        out=eq[:], in0=qfp[:, :, None].to_broadcast([128, 4, 512]),
                        in1=qff[:, None, :].to_broadcast([128, 4, 512]), op=OP.is_equal)
                    count = sm.tile([128, 4], dt.float32)
                    nc.vector.tensor_reduce(out=count[:, :, None], in_=eq[:],
                                            axis=AX.X, op=OP.add)
                    nc.vector.tensor_tensor(
                        out=eq[:], in0=eq[:],
                        in1=candB[:, None, :].to_broadcast([128, 4, 512]), op=OP.mult)
                    dup = sm.tile([128, 4], dt.float32)
                    nc.vector.tensor_reduce(out=dup[:, :, None], in_=eq[:],
                                            axis=AX.X, op=OP.add)

                    mask = sm.tile([128, 4], dt.float32)
                    nc.vector.tensor_scalar(out=mask[:], in0=qfp[:],
                                            scalar1=float(NE), scalar2=None,
                                            op0=OP.is_lt)
                    rc = sm.tile([128, 4], dt.float32)
                    nc.vector.reciprocal(rc[:], count[:])
                    mrc = sm.tile([128, 4], dt.float32)
                    nc.vector.tensor_tensor(out=mrc[:], in0=mask[:], in1=rc[:], op=OP.mult)
                    md = sm.tile([128, 4], dt.float32)
                    nc.vector.tensor_tensor(out=md[:], in0=dup[:], in1=mask[:], op=OP.mult)
                    vmp = sm.tile([128, 1], dt.float32)
                    nc.vector.tensor_reduce(out=vmp[:], in_=md[:], axis=AX.X, op=OP.max)
                    nep = sm.tile([128, 1], dt.float32)
                    nc.vector.tensor_reduce(out=nep[:], in_=mrc[:], axis=AX.X, op=OP.add)

                    # cross-partition reductions via PE transpose
                    tv_ps = tps.tile([128, 128], dt.float32, tag="tp")
                    nc.tensor.transpose(tv_ps[:1, :], vmp[:], ident[:])
                    tv = sm.tile([1, 128], dt.float32)
                    nc.vector.tensor_copy(tv[:], tv_ps[:1, :])
                    vmn = sm.tile([1, 1], dt.float32)
                    nc.vector.tensor_reduce(out=vmn[:], in_=tv[:], axis=AX.X,
                                            op=OP.max, negate=True)   # -vmax
                    tn_ps = tps.tile([128, 128], dt.float32, tag="tp")
                    nc.tensor.transpose(tn_ps[:1, :], nep[:], ident[:])
                    tn = sm.tile([1, 128], dt.float32)
                    nc.vector.tensor_copy(tn[:], tn_ps[:1, :])
                    neff = sm.tile([1, 1], dt.float32)
                    nc.vector.tensor_reduce(out=neff[:], in_=tn[:], axis=AX.X, op=OP.add)

                    vmn_ps = mps.tile([128, 1], dt.float32, tag="mm")
                    nc.tensor.matmul(vmn_ps[:], ones128[:], vmn[:], start=True, stop=True)
                    vmnB = sm.tile([128, 1], dt.float32)
                    nc.vector.tensor_copy(vmnB[:], vmn_ps[:])
                    exd = sm.tile([128, 4], dt.float32)
                    nc.scalar.activation(exd[:], dup[:], ACT.Exp, bias=vmnB[:], scale=1.0)

                    sede = sm.tile([128, 4], dt.float32)
                    nc.vector.tensor_tensor(out=sede[:], in0=mrc[:], in1=exd[:], op=OP.mult)
                    sedp = sm.tile([128, 1], dt.float32)
                    nc.vector.tensor_reduce(out=sedp[:], in_=sede[:], axis=AX.X, op=OP.add)
                    ts_ps = tps.tile([128, 128], dt.float32, tag="tp")
                    nc.tensor.transpose(ts_ps[:1, :], sedp[:], ident[:])
                    tsed = sm.tile([1, 128], dt.float32)
                    nc.vector.tensor_copy(tsed[:], ts_ps[:1, :])
                    sed0 = sm.tile([1, 1], dt.float32)
                    nc.vector.tensor_reduce(out=sed0[:], in_=tsed[:], axis=AX.X, op=OP.add)

                    e_nm = sm.tile([1, 1], dt.float32)
                    nc.scalar.activation(e_nm[:], vmn[:], ACT.Exp)     # exp(-vmax)
                    t1 = sm.tile([1, 1], dt.float32)
                    nc.vector.tensor_scalar(out=t1[:], in0=neff[:], scalar1=-1.0,
                                            scalar2=float(NE), op0=OP.mult, op1=OP.add)
                    d1 = sm.tile([1, 1], dt.float32)
                    nc.vector.tensor_tensor(out=d1[:], in0=t1[:], in1=e_nm[:], op=OP.mult)
                    denom = sm.tile([1, 1], dt.float32)
                    nc.vector.tensor_add(denom[:], d1[:], sed0[:])
                    rden = sm.tile([1, 1], dt.float32)
                    nc.vector.reciprocal(rden[:], denom[:])
                    base = sm.tile([1, 1], dt.float32)
                    nc.vector.tensor_tensor(out=base[:], in0=e_nm[:], in1=rden[:],
                                            op=OP.mult)

                    br2 = sm.tile([1, 2], dt.float32)
                    nc.vector.tensor_copy(br2[:, 0:1], rden[:])
                    nc.vector.tensor_copy(br2[:, 1:2], base[:])
                    bb_ps = mps.tile([128, 2], dt.float32, tag="mm")
                    nc.tensor.matmul(bb_ps[:], ones128[:], br2[:], start=True, stop=True)
                    bb2 = sm.tile([128, 2], dt.float32)
                    nc.vector.tensor_copy(bb2[:], bb_ps[:])

                    outv = sm.tile([128, 4], dt.float32)
                    nc.vector.tensor_tensor(out=outv[:], in0=exd[:],
                                            in1=bb2[:, 0:1].to_broadcast([128, 4]),
                                            op=OP.mult)
                    if phase == 3.8:
                        nc.sync.dma_start(out[:, 0:4], outv[:])

                if phase >= 3.9:
                    fill = sm.tile([128, FILL_W], dt.float32)
                    nc.vector.tensor_copy(fill[:],
                                          bb2[:, 1:2].to_broadcast([128, FILL_W]))
                    for q in range(4):
                        nc.sync.dma_start(out[:, q * FILL_W:(q + 1) * FILL_W], fill[:])
                    tc.strict_bb_all_engine_barrier()
                    out_flat = out[:].rearrange("p f -> (p f)")[:, None]
                    for q in range(4):
                        nc.gpsimd.indirect_dma_start(
                            out=out_flat,
                            out_offset=bass.IndirectOffsetOnAxis(ap=qip[:, q:q + 1], axis=0),
                            in_=outv[:, q:q + 1],
                            in_offset=None)

    nc.compile()
    return nc


def _host_prep(span_embs, triplet_ids_tr, offsets_tr, attention_tr, qid_inds,
               emb_weight, span_W, span_b):
    span_embs = np.asarray(span_embs, dtype=np.float32)
    ids = np.asarray(triplet_ids_tr).astype(np.int64)
    offs = np.asarray(offsets_tr).astype(np.int64)
    att = np.asarray(attention_tr, dtype=np.float32)
    qid = np.asarray(qid_inds).astype(np.int64)
    emb_weight = np.asarray(emb_weight, dtype=np.float32)
    span_W = np.asarray(span_W, dtype=np.float32)
    span_b = np.asarray(span_b, dtype=np.float32)

    # bag id per element (general sorted offsets, offs[b,0] == 0)
    pos = np.arange(L)
    seg = np.empty((B, L), dtype=np.int64)
    for b in range(B):
        seg[b] = np.searchsorted(offs[b], pos, side='right') - 1

    bcol = (np.arange(B)[:, None] * 16 + (seg % 16))        # p = b*16 + m%16
    # device j2 axis: groups contiguous for the s'-softmax
    j2 = ((seg // 16) % 2) * 16 + seg // 32
    k_of = ids // TS
    lid = (ids % TS).astype(np.int64)
    halfsel = (lid >= HALF).astype(np.int64)
    lidx = lid - HALF * halfsel

    # rank within (core, half, partition) group, in stable order
    key = ((k_of * 2 + halfsel) * 128 + bcol).ravel()
    order = np.argsort(key, kind='stable')
    sk = key[order]
    starts = np.r_[0, np.flatnonzero(sk[1:] != sk[:-1]) + 1]
    group_id = np.cumsum(np.r_[0, (sk[1:] != sk[:-1]).astype(np.int64)])
    rank_sorted = np.arange(sk.size) - starts[group_id]
    rank = np.empty(sk.size, dtype=np.int64)
    rank[order] = rank_sorted

    max_rank = int(rank.max())
    nhalf = max(192, ((max_rank + 1 + 7) // 8) * 8)   # per-half slots
    nslot = 2 * nhalf
    chi = nslot // NCH
    gw = chi * 128 // 16

    kf = k_of.ravel()
    pf = bcol.ravel()
    j2f = j2.ravel()
    lf = lidx.ravel()
    af = att.ravel().astype(np.float32)
    cf = halfsel.ravel() * (NCH // 2) + rank // chi   # chunk
    ilocf = rank % chi

    # spansT [768, 128] (col = b*16+s) and spans_all [128, 768]
    spans_all = np.ascontiguousarray(span_embs.reshape(128, E))
    spansT = np.ascontiguousarray(spans_all.T)
    WT = np.ascontiguousarray(emb_weight.T)          # [768, 100000]
    spanw = np.tile(span_W[:, 0][None, :], (128, 1)).astype(np.float32)
    spanb_r = np.full((128, 1), float(span_b[0]), dtype=np.float32)

    r = np.arange(128)
    hostb = (r[:, None] // 16 == r[None, :] // 16).astype(np.float32)
    hostm = (r[:, None] % 16 == np.arange(32)[None, :] % 16).astype(np.float32)

    x = np.arange(512)
    j2d = x % 32
    mx_map = x // 32 + 16 * (2 * (j2d % 16) + j2d // 16)   # position x -> bag m

    in_maps = []
    for k in range(N_CORES):
        sel = kf == k
        p_k, j2_k = pf[sel], j2f[sel]
        l_k, a_k = lf[sel], af[sel]
        c_k, il_k = cf[sel], ilocf[sel]

        gidx_flat = np.zeros((NCH, chi * 128), dtype=np.int16)
        gidx_flat[c_k, il_k * 128 + p_k] = l_k.astype(np.int16)
        gidx = np.zeros((128, NCH * gw), dtype=np.int16)
        for c in range(NCH):
            wrapped = gidx_flat[c].reshape(gw, 16).T       # [16, gw]
            gidx[:, c * gw:(c + 1) * gw] = np.tile(wrapped, (8, 1))

        satt = np.zeros((128, NCH, 32, chi), dtype=np.float32)
        satt[p_k, c_k, j2_k, il_k] = a_k

        own = k
        hostown = np.zeros((128, 16), dtype=np.float32)
        hostown[own * 16 + np.arange(16), np.arange(16)] = 1.0

        qx = qid[own][mx_map]
        in_maps.append(dict(
            wt=np.ascontiguousarray(WT[:, k * TS:(k + 1) * TS]),
            spansT=spansT, spans_all=spans_all, spanw=spanw, spanb=spanb_r,
            gidx=gidx, satt=np.ascontiguousarray(satt.reshape(128, -1)),
            hostb=hostb, hostm=hostm, hostown=hostown,
            qidp_i=qx.reshape(128, 4).astype(np.int32),
            qidp_f=qx.reshape(128, 4).astype(np.float32),
            qidf_free=np.tile(qx[None, :], (128, 1)).astype(np.float32),
        ))
    return in_maps, nslot


def kernel_run(inputs, trace=False):
    in_maps, nslot = _host_prep(**inputs)
    if nslot not in _cache:
        _cache[nslot] = _build(nslot)
    nc = _cache[nslot]
    res = run_bass_kernel_spmd(nc, in_maps, core_ids=list(range(N_CORES)),
                               trace=trace)
    out = np.stack([r["out"].reshape(-1)[:NE] for r in res.results])
    return out[:, :, None].astype(np.float32), res


def kernel(**inputs):
    out, _ = kernel_run(inputs)
    return out



# revision 3
# speedup vs baseline: 1.3284x; 1.3284x over previous
"""Trainium2 Bass kernel for nn_EntityResolution (segment_reduce).

Strategy (8 cores, single launch, one AllReduce):
  - The 307MB embedding table is row-sharded: core k holds rows
    [k*12500, (k+1)*12500) of emb_weight, pre-transposed and cast to
    bf16 on host: wt = W.T shard [768, 12500].
  - Phase 1: V[t, c] = sum_e wt[e, t] * spansT[e, c] for all 128
    columns c = b*16 + s (bf16 matmul, fp32 PSUM). V tiles are DMAed
    straight from PSUM to DRAM (two halves v_a/v_b so phase 2 can
    start when the first half is complete).
  - Phase 2: every element (any batch) whose triplet id falls in shard
    k is processed on core k. Host assigns each element a slot
    (p = b*16 + m%16, chunk, iloc) and ships gidx (int16 quarter-row
    ids: lid*2 + p//64) plus satt (bf16 att one-hot over j2).
    dma_gather pulls 256B quarter rows of V; an ident64-mask
    multiply+reduce extracts V[lid, p]; multiply against satt and
    reduce yields partial sum1 [128 (b,s), 32 (j2)].
  - Phase 3: 16KB AllReduce combines the 8 partial sums.
  - Phase 4: softmax over s' (strided free-axis view), span-score
    multiply (hosted-mask matmul), own-batch extraction, 512-softmax.
  - Phase 5: duplicate summing via is_equal against the host-shipped
    qid table, exp, host-precomputed first-occurrence mask and
    distinct count give the 1M-softmax denominator exactly; output =
    constant fill + one 512-index dma_scatter_add of the deltas.
"""
import sys
sys.path.insert(0, '/opt/trn_rl_repo')

import numpy as np
import ml_dtypes

import concourse.bass as bass
import concourse.bacc as bacc
import concourse.mybir as mybir
import concourse.tile as tile
from concourse import library_config
from concourse.bass_utils import run_bass_kernel_spmd

# problem shapes (hardcoded; kernel.py must be self-contained)
B, S, C, PB, E = 8, 16, 32, 64, 768
M = S * C                # 512 bags per batch
L = M * PB               # 32768 triplet ids per batch
T = 100000               # triplet vocab
NE = 1000000             # entities
N_CORES = 8
TS = T // N_CORES        # 12500 shard rows
HALF = 6250              # v table split
NCH = 16                 # gather chunks (8 per half)
MACRO = 1250             # wt macro tile (10 matmul tiles of 125)
NMAC = TS // MACRO       # 10 macros, 5 per half
OUT_W = 7872             # out [128, 7872] -> flat 1007616 >= NE+1
FILL_W = OUT_W // 4
SROWS = 64               # scatter elem size (fp32) = 256B

# hconst packing offsets (fp32 columns)
HC_SPANS = 0             # [768]  span_embs row-major per p
HC_SPANW = 768           # [768]  span_W replicated
HC_HOSTB = 1536          # [128]  same-batch mask
HC_HOSTM = 1664          # [32]   j2%16 == p%16 mask
HC_HOSTOWN = 1696        # [16]   own-batch extraction
HC_ID64 = 1712           # [64]   ident64: (q == p%64)
HC_QIDP = 1776           # [4]    qid per slot
HC_QIDF = 1780           # [512]  qid table
HC_FIRST = 2292          # [4]    first-occurrence mask
HC_NBASE = 2296          # [1]    NE - n_distinct_valid
HC_OH64 = 2297           # [256]  onehot64 per slot
HC_SPANB = 2553          # [1]    span_b
HC_W = 2554

AX = mybir.AxisListType
OP = mybir.AluOpType
ACT = mybir.ActivationFunctionType
dt = mybir.dt

_cache = {}


def _build(nslot):
    chi = nslot // NCH               # slots per partition per chunk
    nidx = chi * 128                 # idxs per dma_gather chunk
    gw = nidx // 16                  # wrapped idx width per chunk
    nc = bacc.Bacc("TRN2", target_bir_lowering=False, debug=False,
                   num_devices=N_CORES)

    wt = nc.dram_tensor("wt", [E, TS], dt.bfloat16, kind="ExternalInput")
    spansT6 = nc.dram_tensor("spansT6", [128, 6 * 128], dt.bfloat16,
                             kind="ExternalInput")
    hconst = nc.dram_tensor("hconst", [128, HC_W], dt.float32,
                            kind="ExternalInput")
    sidx = nc.dram_tensor("sidx", [128, 32], dt.int16, kind="ExternalInput")
    gidx = nc.dram_tensor("gidx", [128, NCH * gw], dt.int16,
                          kind="ExternalInput")
    satt = nc.dram_tensor("satt", [128, NCH * 32 * chi], dt.bfloat16,
                          kind="ExternalInput")
    out = nc.dram_tensor("out", [128, OUT_W], dt.float32,
                         kind="ExternalOutput")

    rg = [list(range(N_CORES))]

    with tile.TileContext(nc) as tc:
        with (
            tc.tile_pool(name="wtp", bufs=2) as wtp,
            tc.tile_pool(name="gp", bufs=2) as gp,
            tc.tile_pool(name="wk", bufs=2) as wk,
            tc.tile_pool(name="sb", bufs=1) as sb,
            tc.tile_pool(name="sm", bufs=1) as sm,
            tc.tile_pool(name="vps", bufs=2, space="PSUM") as vps,
            tc.tile_pool(name="tps", bufs=2, space="PSUM") as tps,
            tc.tile_pool(name="mps", bufs=1, space="PSUM") as mps,
            tc.tile_pool(name="dram", bufs=1, space="DRAM") as dram,
        ):
            nc.gpsimd.load_library(library_config.mlp)

            # resident inputs
            spansT_sb = sb.tile([128, 6, 128], dt.bfloat16)
            nc.sync.dma_start(
                spansT_sb[:], spansT6[:].rearrange("p (a b) -> p a b", a=6))
            gidx_sb = sb.tile([128, NCH * gw], dt.int16)
            nc.sync.dma_start(gidx_sb[:], gidx[:])
            satt_sb = sb.tile([128, NCH, 32, chi], dt.bfloat16)
            nc.sync.dma_start(
                satt_sb[:],
                satt[:].rearrange("p (c a b) -> p c a b", c=NCH, a=32))
            hc = sb.tile([128, HC_W], dt.float32)
            nc.sync.dma_start(hc[:], hconst[:])
            sidx_sb = sb.tile([128, 32], dt.int16)
            nc.sync.dma_start(sidx_sb[:], sidx[:])

            # ---------- phase 1: V = W @ spans, PE-transposed to [t, c] ----
            v_a = dram.tile([HALF, 128], dt.float32)
            v_b = dram.tile([TS - HALF, 128], dt.float32)
            for mac in range(NMAC):
                moff = mac * MACRO
                w_t = wtp.tile([128, 6, MACRO], dt.bfloat16, tag="wt")
                nc.sync.dma_start(
                    w_t[:],
                    wt[:, moff:moff + MACRO].rearrange(
                        "(a p) t -> p a t", p=128))
                vdst = v_a if mac < NMAC // 2 else v_b
                ro = moff if mac < NMAC // 2 else moff - HALF
                for hgrp in range(2):
                    v_ps = vps.tile([125, 5, 128], dt.float32, tag="vps")
                    for ti in range(5):
                        tg = hgrp * 5 + ti
                        for e in range(6):
                            nc.tensor.matmul(
                                v_ps[:, ti, :],
                                w_t[:, e, tg * 125:(tg + 1) * 125],
                                spansT_sb[:, e, :],
                                start=(e == 0), stop=(e == 5))
                    v_sb = wk.tile([125, 5, 128], dt.float32, tag="vsb")
                    nc.scalar.copy(v_sb[:], v_ps[:])
                    nc.scalar.dma_start(
                        vdst[ro + hgrp * 625: ro + (hgrp + 1) * 625, :]
                        .rearrange("(ti t) p -> t ti p", ti=5),
                        v_sb[:])

            # ---------- phase 2: gather quarter-rows + extract + segment ----
            psum1 = sb.tile([128, 32], dt.float32)
            nc.vector.memset(psum1[:], 0.0)
            ident64 = hc[:, HC_ID64:HC_ID64 + 64]
            for c in range(NCH):
                vsrc = v_a if c < NCH // 2 else v_b
                g_t = gp.tile([128, chi, 64], dt.float32, tag="G")
                nc.gpsimd.dma_gather(
                    out_ap=g_t[:],
                    in_ap=vsrc[:].rearrange("r (t c) -> (r t) c", c=64),
                    idxs_ap=gidx_sb[:, c * gw:(c + 1) * gw],
                    num_idxs=nidx, num_idxs_reg=nidx, elem_size=64,
                    single_packet=False)
                nc.vector.tensor_tensor(
                    out=g_t[:], in0=g_t[:],
                    in1=ident64[:, None, :].to_broadcast([128, chi, 64]),
                    op=OP.mult)
                v1 = wk.tile([128, chi], dt.float32, tag="V1")
                nc.vector.tensor_reduce(out=v1[:, :, None], in_=g_t[:],
                                        axis=AX.X, op=OP.add)
                v1b = wk.tile([128, chi], dt.bfloat16, tag="V1B")
                nc.vector.tensor_copy(v1b[:], v1[:])
                t2 = wk.tile([128, 32, chi], dt.bfloat16, tag="T2")
                nc.vector.tensor_tensor(
                    out=t2[:], in0=satt_sb[:, c, :, :],
                    in1=v1b[:, None, :].to_broadcast([128, 32, chi]),
                    op=OP.mult)
                psc = wk.tile([128, 32], dt.float32, tag="PSC")
                nc.vector.tensor_reduce(out=psc[:, :, None], in_=t2[:],
                                        axis=AX.X, op=OP.add)
                nc.vector.tensor_add(psum1[:], psum1[:], psc[:])

            # ---------- phase 3: AllReduce [128, 32] -----------------------
            ar_in = dram.tile([128, 32], dt.float32)
            ar_out = dram.tile([128, 32], dt.float32)
            nc.gpsimd.dma_start(ar_in[:], psum1[:])
            nc.gpsimd.collective_compute(
                "AllReduce", OP.add, replica_groups=rg,
                ins=[ar_in.opt()], outs=[ar_out.opt()])
            sum1 = sm.tile([128, 32], dt.float32)
            nc.gpsimd.dma_start(sum1[:], ar_out[:])

            # ---------- phase 4: softmaxes ---------------------------------
            # span scores: ssc[p] = span_embs[p] . span_W + b
            tmp768 = sm.tile([128, E], dt.float32)
            nc.vector.tensor_tensor(out=tmp768[:],
                                    in0=hc[:, HC_SPANS:HC_SPANS + E],
                                    in1=hc[:, HC_SPANW:HC_SPANW + E],
                                    op=OP.mult)
            ssc = sm.tile([128, 1], dt.float32)
            nc.vector.tensor_reduce(out=ssc[:], in_=tmp768[:], axis=AX.X,
                                    op=OP.add)
            nc.vector.tensor_add(ssc[:], ssc[:],
                                 hc[:, HC_SPANB:HC_SPANB + 1])

            # softmax over s (strided view [128, 2, 16])
            def v216(ap):
                return ap.rearrange("p (two s2) -> p two s2", two=2)
            mx = sm.tile([128, 2], dt.float32)
            nc.vector.tensor_reduce(out=mx[:, :, None], in_=v216(sum1[:]),
                                    axis=AX.X, op=OP.max)
            e1 = sm.tile([128, 32], dt.float32)
            nc.vector.tensor_tensor(
                out=v216(e1[:]), in0=v216(sum1[:]),
                in1=mx[:, :, None].to_broadcast([128, 2, 16]),
                op=OP.subtract)
            nc.scalar.activation(e1[:], e1[:], ACT.Exp)
            smsum = sm.tile([128, 2], dt.float32)
            nc.vector.tensor_reduce(out=smsum[:, :, None], in_=v216(e1[:]),
                                    axis=AX.X, op=OP.add)
            rsm = sm.tile([128, 2], dt.float32)
            nc.vector.reciprocal(rsm[:], smsum[:])
            nc.vector.tensor_tensor(
                out=v216(e1[:]), in0=v216(e1[:]),
                in1=rsm[:, :, None].to_broadcast([128, 2, 16]), op=OP.mult)

            # SSB[p, j2] = span_score(b(p), j2%16) via hosted-mask matmul
            rhsb = sm.tile([128, 32], dt.float32)
            nc.vector.tensor_tensor(out=rhsb[:],
                                    in0=hc[:, HC_HOSTM:HC_HOSTM + 32],
                                    in1=ssc[:].to_broadcast([128, 32]),
                                    op=OP.mult)
            ssb_ps = mps.tile([128, 32], dt.float32, tag="mm")
            nc.tensor.matmul(ssb_ps[:], hc[:, HC_HOSTB:HC_HOSTB + 128],
                             rhsb[:], start=True, stop=True)
            mult2 = sm.tile([128, 32], dt.float32)
            nc.vector.tensor_tensor(out=mult2[:], in0=e1[:], in1=ssb_ps[:],
                                    op=OP.mult)

            # own-batch extraction -> [16, 32] -> [1, 512]
            own_ps = mps.tile([16, 32], dt.float32, tag="mm")
            nc.tensor.matmul(own_ps[:], hc[:, HC_HOSTOWN:HC_HOSTOWN + 16],
                             mult2[:], start=True, stop=True)
            own = sm.tile([16, 32], dt.float32)
            nc.vector.tensor_copy(own[:], own_ps[:])
            cn = sm.tile([1, 512], dt.float32)
            nc.sync.dma_start(cn[:].rearrange("p (a bb) -> p a bb", a=16),
                              own[:])

            # softmax over 512
            mxn = sm.tile([1, 1], dt.float32)
            nc.vector.tensor_reduce(out=mxn[:], in_=cn[:], axis=AX.X,
                                    op=OP.max, negate=True)
            e5 = sm.tile([1, 512], dt.float32)
            nc.scalar.activation(e5[:], cn[:], ACT.Exp, bias=mxn[:],
                                 scale=1.0)
            s5 = sm.tile([1, 1], dt.float32)
            nc.vector.tensor_reduce(out=s5[:], in_=e5[:], axis=AX.X,
                                    op=OP.add)
            r5 = sm.tile([1, 1], dt.float32)
            nc.vector.reciprocal(r5[:], s5[:])
            cand = sm.tile([1, 512], dt.float32)
            nc.vector.tensor_tensor(out=cand[:], in0=e5[:],
                                    in1=r5[:].to_broadcast([1, 512]),
                                    op=OP.mult)

            # ---------- phase 5: dup sums + exact denominator + scatter ----
            ones128 = sm.tile([1, 128], dt.float32)
            nc.vector.memset(ones128[:], 1.0)
            cb_ps = mps.tile([128, 512], dt.float32, tag="mm")
            nc.tensor.matmul(cb_ps[:], ones128[:], cand[:], start=True,
                             stop=True)
            candB = sm.tile([128, 512], dt.float32)
            nc.vector.tensor_copy(candB[:], cb_ps[:])

            eq = sm.tile([128, 4, 512], dt.float32)
            nc.vector.tensor_tensor(
                out=eq[:],
                in0=hc[:, HC_QIDP:HC_QIDP + 4, None].to_broadcast(
                    [128, 4, 512]),
                in1=hc[:, None, HC_QIDF:HC_QIDF + 512].to_broadcast(
                    [128, 4, 512]),
                op=OP.is_equal)
            nc.vector.tensor_tensor(
                out=eq[:], in0=eq[:],
                in1=candB[:, None, :].to_broadcast([128, 4, 512]),
                op=OP.mult)
            dup = sm.tile([128, 4], dt.float32)
            nc.vector.tensor_reduce(out=dup[:, :, None], in_=eq[:],
                                    axis=AX.X, op=OP.add)

            # exp(dup) (dup <= 1, no stabilization needed)
            ex = sm.tile([128, 4], dt.float32)
            nc.scalar.activation(ex[:], dup[:], ACT.Exp)
            exm = sm.tile([128, 4], dt.float32)
            nc.vector.tensor_tensor(out=exm[:], in0=ex[:],
                                    in1=hc[:, HC_FIRST:HC_FIRST + 4],
                                    op=OP.mult)
            sred = sm.tile([128, 1], dt.float32)
            nc.vector.tensor_reduce(out=sred[:], in_=exm[:], axis=AX.X,
                                    op=OP.add)
            # cross-partition total via PE transpose
            idn = sb.tile([128, 128], dt.float32)
            from concourse.masks import make_identity
            make_identity(nc, idn[:])
            ts_ps = tps.tile([128, 128], dt.float32, tag="tp")
            nc.tensor.transpose(ts_ps[:1, :], sred[:], idn[:])
            tsed = sm.tile([1, 128], dt.float32)
            nc.vector.tensor_copy(tsed[:], ts_ps[:1, :])
            sed0 = sm.tile([1, 1], dt.float32)
            nc.vector.tensor_reduce(out=sed0[:], in_=tsed[:], axis=AX.X,
                                    op=OP.add)
            denom = sm.tile([1, 1], dt.float32)
            nc.vector.tensor_add(denom[:], sed0[:],
                                 hc[0:1, HC_NBASE:HC_NBASE + 1])
            rden = sm.tile([1, 1], dt.float32)
            nc.vector.reciprocal(rden[:], denom[:])
            bb_ps = mps.tile([128, 1], dt.float32, tag="mm")
            nc.tensor.matmul(bb_ps[:], ones128[:], rden[:], start=True,
                             stop=True)
            rdenB = sm.tile([128, 1], dt.float32)
            nc.vector.tensor_copy(rdenB[:], bb_ps[:])

            # base fill of the whole output
            fill = sm.tile([128, FILL_W], dt.float32)
            nc.vector.tensor_copy(fill[:],
                                  rdenB[:].to_broadcast([128, FILL_W]))
            for q in range(4):
                nc.sync.dma_start(out[:, q * FILL_W:(q + 1) * FILL_W],
                                  fill[:])

            # delta = firstocc * (exp(dup) - 1) / denom; src = delta x onehot
            exm1 = sm.tile([128, 4], dt.float32)
            nc.vector.tensor_scalar(out=exm1[:], in0=ex[:], scalar1=-1.0,
                                    scalar2=None, op0=OP.add)
            nc.vector.tensor_tensor(out=exm1[:], in0=exm1[:],
                                    in1=hc[:, HC_FIRST:HC_FIRST + 4],
                                    op=OP.mult)
            nc.vector.tensor_tensor(out=exm1[:], in0=exm1[:],
                                    in1=rdenB[:].to_broadcast([128, 4]),
                                    op=OP.mult)
            src = sm.tile([128, 4, SROWS], dt.float32)
            nc.vector.tensor_tensor(
                out=src[:],
                in0=exm1[:, :, None].to_broadcast([128, 4, SROWS]),
                in1=hc[:, HC_OH64:HC_OH64 + 256].rearrange(
                    "p (a c) -> p a c", a=4),
                op=OP.mult)

            tc.strict_bb_all_engine_barrier()
            nc.gpsimd.dma_scatter_add(
                out_ap=out[:].rearrange("p f -> (p f)").rearrange(
                    "(r c) -> r c", c=SROWS),
                in_ap=src[:],
                idxs_ap=sidx_sb[:],
                num_idxs=512, num_idxs_reg=512, elem_size=SROWS,
                single_packet=False)

    nc.compile()
    return nc


def _host_prep(span_embs, triplet_ids_tr, offsets_tr, attention_tr, qid_inds,
               emb_weight, span_W, span_b):
    span_embs = np.asarray(span_embs, dtype=np.float32)
    ids = np.asarray(triplet_ids_tr).astype(np.int64)
    offs = np.asarray(offsets_tr).astype(np.int64)
    att = np.asarray(attention_tr, dtype=np.float32)
    qid = np.asarray(qid_inds).astype(np.int64)
    emb_weight = np.asarray(emb_weight, dtype=np.float32)
    span_W = np.asarray(span_W, dtype=np.float32)
    span_b = np.asarray(span_b, dtype=np.float32)

    # bag id per element (general sorted offsets, offs[b,0] == 0)
    pos = np.arange(L)
    seg = np.empty((B, L), dtype=np.int64)
    for b in range(B):
        seg[b] = np.searchsorted(offs[b], pos, side='right') - 1

    bcol = (np.arange(B)[:, None] * 16 + (seg % 16))        # p = b*16 + m%16
    # device j2 axis: j2 = (c//16)*16 + s, groups contiguous for softmax
    j2 = ((seg // 16) % 2) * 16 + seg // 32
    k_of = ids // TS
    lid = (ids % TS).astype(np.int64)
    halfsel = (lid >= HALF).astype(np.int64)
    lidx = lid - HALF * halfsel

    # rank within (core, half, partition) group, in stable order
    key = ((k_of * 2 + halfsel) * 128 + bcol).ravel()
    order = np.argsort(key, kind='stable')
    sk = key[order]
    starts = np.r_[0, np.flatnonzero(sk[1:] != sk[:-1]) + 1]
    group_id = np.cumsum(np.r_[0, (sk[1:] != sk[:-1]).astype(np.int64)])
    rank_sorted = np.arange(sk.size) - starts[group_id]
    rank = np.empty(sk.size, dtype=np.int64)
    rank[order] = rank_sorted

    max_rank = int(rank.max())
    chi = max(8, (max_rank + 8) // 8)     # slots per partition per chunk
    nslot = NCH * chi
    gw = chi * 128 // 16

    kf = k_of.ravel()
    pf = bcol.ravel()
    j2f = j2.ravel()
    lf = lidx.ravel()
    af = att.ravel().astype(np.float32)
    cf = halfsel.ravel() * (NCH // 2) + rank // chi   # chunk
    ilocf = rank % chi

    # spans layouts
    spans_all = np.ascontiguousarray(span_embs.reshape(128, E))
    spansT = np.ascontiguousarray(spans_all.T)               # [768, 128]
    spansT6 = np.ascontiguousarray(
        spansT.reshape(6, 128, 128).transpose(1, 0, 2).reshape(128, 768)
    ).astype(ml_dtypes.bfloat16)
    WT = np.ascontiguousarray(emb_weight.T)                  # [768, 100000]

    r = np.arange(128)
    hostb = (r[:, None] // 16 == r[None, :] // 16).astype(np.float32)
    hostm = (r[:, None] % 16 == np.arange(32)[None, :] % 16).astype(
        np.float32)
    ident64 = (np.arange(64)[None, :] == (r % 64)[:, None]).astype(
        np.float32)

    x = np.arange(512)
    j2d = x % 32
    mx_map = x // 32 + 16 * (2 * (j2d % 16) + j2d // 16)   # position x -> bag

    in_maps = []
    for k in range(N_CORES):
        sel = kf == k
        p_k, j2_k = pf[sel], j2f[sel]
        l_k, a_k = lf[sel], af[sel]
        c_k, il_k = cf[sel], ilocf[sel]

        # gather idxs: quarter-row id = lid*2 + p//64
        gidx_flat = np.zeros((NCH, chi * 128), dtype=np.int16)
        gidx_flat[c_k, il_k * 128 + p_k] = (l_k * 2 + p_k // 64).astype(
            np.int16)
        gidx = np.zeros((128, NCH * gw), dtype=np.int16)
        for c in range(NCH):
            wrapped = gidx_flat[c].reshape(gw, 16).T       # [16, gw]
            gidx[:, c * gw:(c + 1) * gw] = np.tile(wrapped, (8, 1))

        satt = np.zeros((128, NCH, 32, chi), dtype=np.float32)
        satt[p_k, c_k, j2_k, il_k] = a_k

        own = k
        hostown = np.zeros((128, 16), dtype=np.float32)
        hostown[own * 16 + np.arange(16), np.arange(16)] = 1.0

        qx = qid[own][mx_map]                               # [512]
        valid = qx < NE
        firstocc = np.zeros(512, dtype=np.float32)
        _, first_idx = np.unique(qx, return_index=True)
        for fi in first_idx:
            if valid[fi]:
                firstocc[fi] = 1.0
        n_distinct = int(np.unique(qx[valid]).size)
        nbase = float(NE - n_distinct)

        onehot64 = np.zeros((512, SROWS), dtype=np.float32)
        onehot64[np.arange(512), qx % SROWS] = 1.0

        # scatter idx list position i = q*128 + p  ->  slot (p, q)
        i = np.arange(512)
        pos_i = (i % 128) * 4 + (i // 128)
        sidx_flat = (qx[pos_i] // SROWS).astype(np.int16)
        sidx = np.tile(sidx_flat.reshape(32, 16).T, (8, 1))  # [128, 32]

        hconst = np.zeros((128, HC_W), dtype=np.float32)
        hconst[:, HC_SPANS:HC_SPANS + E] = spans_all
        hconst[:, HC_SPANW:HC_SPANW + E] = span_W[:, 0][None, :]
        hconst[:, HC_HOSTB:HC_HOSTB + 128] = hostb
        hconst[:, HC_HOSTM:HC_HOSTM + 32] = hostm
        hconst[:, HC_HOSTOWN:HC_HOSTOWN + 16] = hostown
        hconst[:, HC_ID64:HC_ID64 + 64] = ident64
        hconst[:, HC_QIDP:HC_QIDP + 4] = qx.reshape(128, 4)
        hconst[:, HC_QIDF:HC_QIDF + 512] = qx[None, :]
        hconst[:, HC_FIRST:HC_FIRST + 4] = firstocc.reshape(128, 4)
        hconst[:, HC_NBASE] = nbase
        hconst[:, HC_OH64:HC_OH64 + 256] = onehot64.reshape(128, 256)
        hconst[:, HC_SPANB] = float(span_b[0])

        in_maps.append(dict(
            wt=np.ascontiguousarray(
                WT[:, k * TS:(k + 1) * TS]).astype(ml_dtypes.bfloat16),
            spansT6=spansT6,
            hconst=hconst,
            sidx=sidx,
            gidx=gidx,
            satt=np.ascontiguousarray(
                satt.reshape(128, -1)).astype(ml_dtypes.bfloat16),
        ))
    return in_maps, nslot


def kernel_run(inputs, trace=False):
    in_maps, nslot = _host_prep(**inputs)
    if nslot not in _cache:
        _cache[nslot] = _build(nslot)
    nc = _cache[nslot]
    res = run_bass_kernel_spmd(nc, in_maps, core_ids=list(range(N_CORES)),
                               trace=trace)
    out = np.stack([r["out"].reshape(-1)[:NE] for r in res.results])
    return out[:, :, None].astype(np.float32), res


def kernel(**inputs):
    out, _ = kernel_run(inputs)
    return out


# revision 27
# speedup vs baseline: 1.4096x; 1.0611x over previous
"""Trainium2 Bass kernel for nn_EntityResolution (segment_reduce).

Strategy (8 cores, single launch, two half ReduceScatters):
  - The 307MB embedding table is row-sharded: core k holds rows
    [k*12500, (k+1)*12500) of emb_weight, pre-transposed and cast to
    bf16 on host: wt = W.T shard [768, 12500].
  - Phase 1: V[t, c] = sum_e wt[e, t] * spansT[e, c] for all 128
    columns c = b*16 + s (bf16 matmul, fp32 PSUM, PSUM->SBUF on the
    Activation engine, macro-consolidated DMAs to DRAM in two halves
    v_a/v_b so phase 2 starts when the first half is complete).
  - Phase 2: every element (any batch) whose triplet id falls in shard
    k is processed on core k. Host assigns each element a slot
    (p = b*16 + m%16, chunk, iloc) and ships gidx (int16 quarter-row
    ids: lid*2 + p//64) plus satt (bf16 att one-hot over j2).
    dma_gather pulls 256B quarter rows of V; an ident64-mask
    multiply+reduce on DVE extracts V[lid, p]; the satt multiply runs
    on the Pool engine and a DVE reduce yields partial sum1
    [128 (b,s), 32 (j2)].
  - Phase 3: per-half ReduceScatter [128,32]->[16,32] (own batch rows);
    the first launches while the second half of phase 2 still runs.
  - Phase 4: softmax over s (strided free-axis view), span-score
    multiply via hosted-mask matmuls, 512-softmax — all on the own
    [16, 32] block; exp without max-shift (values are small).
  - Phase 5: duplicate summing via a precomputed is_equal mask against
    the host-shipped qid table; host-precomputed first-occurrence mask
    and distinct count give the 1M-softmax denominator exactly
    (cross-partition total via gpsimd partition_all_reduce); output =
    constant base fill + one 512-index dma_scatter_add of the deltas.
"""
import sys
sys.path.insert(0, '/opt/trn_rl_repo')

import numpy as np
import ml_dtypes

import concourse.bass as bass
import concourse.bass_isa as bass_isa
import concourse.bacc as bacc
import concourse.mybir as mybir
import concourse.tile as tile
from concourse import library_config
from concourse.bass_utils import run_bass_kernel_spmd

# problem shapes (hardcoded; kernel.py must be self-contained)
B, S, C, PB, E = 8, 16, 32, 64, 768
M = S * C                # 512 bags per batch
L = M * PB               # 32768 triplet ids per batch
T = 100000               # triplet vocab
NE = 1000000             # entities
N_CORES = 8
TS = T // N_CORES        # 12500 shard rows
HALF = 6250              # v table split
NCH = 16                 # gather chunks (8 per half)
MACRO = 1250             # wt macro tile (10 matmul tiles of 125)
NMAC = TS // MACRO       # 10 macros, 5 per half
OUT_W = 7872             # out [128, 7872] -> flat 1007616 >= NE+1
FILL_W = OUT_W // 4
SROWS = 64               # scatter elem size (fp32) = 256B

# hconst packing offsets (fp32 columns)
HC_SPANS = 0             # [768]  span_embs row-major per p
HC_SPANW = 768           # [768]  span_W replicated
HC_HOSTB = 1536          # [128]  (unused; kept for layout stability)
HC_HOSTM = 1664          # [32]   j2%16 == p%16 mask
HC_BLK = 1696            # [16]   own-batch block mask (p//16 == own)
HC_ID64 = 1712           # [64]   ident64: (q == p%64)
HC_QIDP = 1776           # [4]    qid per slot
HC_QIDF = 1780           # [512]  qid table
HC_FIRST = 2292          # [4]    first-occurrence mask
HC_NBASE = 2296          # [1]    NE - n_distinct_valid
HC_OH64 = 2297           # [256]  onehot64 per slot
HC_SPANB = 2553          # [1]    span_b
HC_W = 2554

AX = mybir.AxisListType
OP = mybir.AluOpType
ACT = mybir.ActivationFunctionType
dt = mybir.dt

_cache = {}


def _build(nslot):
    chi = nslot // NCH               # slots per partition per chunk
    nidx = chi * 128                 # idxs per dma_gather chunk
    gw = nidx // 16                  # wrapped idx width per chunk
    nc = bacc.Bacc("TRN2", target_bir_lowering=False, debug=False,
                   num_devices=N_CORES)

    wt = nc.dram_tensor("wt", [E, TS], dt.bfloat16, kind="ExternalInput")
    spansT6 = nc.dram_tensor("spansT6", [128, 6 * 128], dt.bfloat16,
                             kind="ExternalInput")
    hconst = nc.dram_tensor("hconst", [128, HC_W], dt.float32,
                            kind="ExternalInput")
    sidx = nc.dram_tensor("sidx", [128, 32], dt.int16, kind="ExternalInput")
    gidx = nc.dram_tensor("gidx", [128, NCH * gw], dt.int16,
                          kind="ExternalInput")
    satt = nc.dram_tensor("satt", [128, NCH * 32 * chi], dt.bfloat16,
                          kind="ExternalInput")
    out = nc.dram_tensor("out", [128, OUT_W], dt.float32,
                         kind="ExternalOutput")

    rg = [list(range(N_CORES))]

    with tile.TileContext(nc) as tc:
        with (
            tc.tile_pool(name="wtp", bufs=4) as wtp,
            tc.tile_pool(name="gp", bufs=2) as gp,
            tc.tile_pool(name="wk", bufs=2) as wk,
            tc.tile_pool(name="sb", bufs=1) as sb,
            tc.tile_pool(name="sm", bufs=1) as sm,
            tc.tile_pool(name="vps", bufs=2, space="PSUM") as vps,
            tc.tile_pool(name="mps", bufs=1, space="PSUM") as mps,
            tc.tile_pool(name="dram", bufs=1, space="DRAM") as dram,
        ):
            nc.gpsimd.load_library(library_config.mlp)

            def load_wt(mac):
                w_t = wtp.tile([128, 6, MACRO], dt.bfloat16, tag="wt")
                nc.sync.dma_start(
                    w_t[:],
                    wt[:, mac * MACRO:(mac + 1) * MACRO].rearrange(
                        "(a p) t -> p a t", p=128))
                return w_t

            # spansT first (needed by the first matmul), then the first two
            # wt macros, then the small resident inputs, so the critical
            # phase-1 pipeline starts immediately.
            spansT_sb = sb.tile([128, 6, 128], dt.bfloat16)
            nc.sync.dma_start(
                spansT_sb[:], spansT6[:].rearrange("p (a b) -> p a b", a=6))
            w_cur = load_wt(0)
            w_nxt = load_wt(1)
            gidx_sb = sb.tile([128, NCH * gw], dt.int16)
            nc.sync.dma_start(gidx_sb[:], gidx[:])
            hc = sb.tile([128, HC_W], dt.float32)
            sidx_sb = sb.tile([128, 32], dt.int16)
            satt_sb = sb.tile([128, NCH, 32, chi], dt.bfloat16)

            def load_residents(mac):
                # deferred off the phase-1 critical path; needed ~when v_a
                # completes
                if mac == 0:
                    nc.sync.dma_start(hc[:], hconst[:])
                elif mac == 1:
                    nc.sync.dma_start(
                        satt_sb[:],
                        satt[:].rearrange("p (c a b) -> p c a b",
                                          c=NCH, a=32))
                elif mac == 2:
                    nc.sync.dma_start(sidx_sb[:], sidx[:])
            # ---------- phase 1: V = W @ spans, PE-transposed to [t, c] ----
            v_a = dram.tile([HALF, 128], dt.float32)
            v_b = dram.tile([TS - HALF, 128], dt.float32)
            for mac in range(NMAC):
                moff = mac * MACRO
                w_t = w_cur
                vdst = v_a if mac < NMAC // 2 else v_b
                ro = moff if mac < NMAC // 2 else moff - HALF
                for hgrp in range(2):
                    v_ps = vps.tile([125, 5, 128], dt.float32, tag="vps")
                    for ti in range(5):
                        tg = hgrp * 5 + ti
                        for e in range(6):
                            nc.tensor.matmul(
                                v_ps[:, ti, :],
                                w_t[:, e, tg * 125:(tg + 1) * 125],
                                spansT_sb[:, e, :],
                                start=(e == 0), stop=(e == 5))
                    v_sb = wk.tile([125, 5, 128], dt.float32, tag="vsb")
                    nc.scalar.copy(v_sb[:], v_ps[:])
                    nc.scalar.dma_start(
                        vdst[ro + hgrp * 625: ro + (hgrp + 1) * 625, :]
                        .rearrange("(ti t) p -> t ti p", ti=5),
                        v_sb[:])
                w_cur = w_nxt
                if mac + 2 < NMAC:
                    w_nxt = load_wt(mac + 2)
                load_residents(mac)

            # independent of phases 1-3: runs in the DVE idle window
            # before the first gather lands
            tmp768 = sm.tile([128, E], dt.float32)
            nc.vector.tensor_tensor(out=tmp768[:],
                                    in0=hc[:, HC_SPANS:HC_SPANS + E],
                                    in1=hc[:, HC_SPANW:HC_SPANW + E],
                                    op=OP.mult)
            ssc = sm.tile([128, 1], dt.float32)
            nc.vector.tensor_reduce(out=ssc[:], in_=tmp768[:], axis=AX.X,
                                    op=OP.add)
            nc.vector.tensor_add(ssc[:], ssc[:],
                                 hc[:, HC_SPANB:HC_SPANB + 1])
            # ssb2[a, j2] = ssc[own*16 + j2%16] via two hosted-mask matmuls
            rhsb = sm.tile([128, 32], dt.float32)
            nc.vector.tensor_tensor(out=rhsb[:],
                                    in0=hc[:, HC_HOSTM:HC_HOSTM + 32],
                                    in1=ssc[:].to_broadcast([128, 32]),
                                    op=OP.mult)
            ssb_ps = mps.tile([16, 32], dt.float32, tag="mm")
            nc.tensor.matmul(ssb_ps[:], hc[:, HC_BLK:HC_BLK + 16],
                             rhsb[:], start=True, stop=True)
            eqm = sm.tile([128, 4, 512], dt.float32)
            nc.vector.tensor_tensor(
                out=eqm[:],
                in0=hc[:, HC_QIDP:HC_QIDP + 4, None].to_broadcast(
                    [128, 4, 512]),
                in1=hc[:, None, HC_QIDF:HC_QIDF + 512].to_broadcast(
                    [128, 4, 512]),
                op=OP.is_equal)

            # ---------- phase 2: gather quarter-rows + extract + segment ----
            # Work split: DVE does mask/extract/psc-reduce, Pool does the
            # satt multiply and psum accumulate, so the two engines pipeline.
            psums = []
            ident64 = hc[:, HC_ID64:HC_ID64 + 64]
            ar_ins, ar_outs = [], []
            for half in range(2):
                psum1 = sb.tile([128, 32], dt.float32)
                nc.vector.memset(psum1[:], 0.0)
                psums.append(psum1)
                for c in range(half * 8, half * 8 + 8):
                    vsrc = v_a if c < NCH // 2 else v_b
                    g_t = gp.tile([128, chi, 64], dt.float32, tag="G")
                    nc.gpsimd.dma_gather(
                        out_ap=g_t[:],
                        in_ap=vsrc[:].rearrange("r (t c) -> (r t) c", c=64),
                        idxs_ap=gidx_sb[:, c * gw:(c + 1) * gw],
                        num_idxs=nidx, num_idxs_reg=nidx, elem_size=64,
                        single_packet=False)
                    nc.vector.tensor_tensor(
                        out=g_t[:], in0=g_t[:],
                        in1=ident64[:, None, :].to_broadcast([128, chi, 64]),
                        op=OP.mult)
                    v1 = wk.tile([128, chi], dt.bfloat16, tag="V1")
                    with nc.allow_low_precision(reason="single nonzero"):
                        nc.vector.tensor_reduce(out=v1[:, :, None],
                                                in_=g_t[:],
                                                axis=AX.X, op=OP.add)
                    t2 = wk.tile([128, 32, chi], dt.bfloat16, tag="T2")
                    nc.vector.tensor_tensor(
                        out=t2[:], in0=satt_sb[:, c, :, :],
                        in1=v1[:, None, :].to_broadcast([128, 32, chi]),
                        op=OP.mult)
                    psc = wk.tile([128, 32], dt.float32, tag="PSC")
                    nc.vector.tensor_reduce(out=psc[:, :, None], in_=t2[:],
                                            axis=AX.X, op=OP.add)
                    nc.vector.tensor_add(psum1[:], psum1[:], psc[:])
                # half-collective: the first launches while the second half
                # of phase 2 is still running, hiding its latency
                ar_in = dram.tile([128, 32], dt.float32)
                ar_out = dram.tile([16, 32], dt.float32)
                nc.gpsimd.dma_start(ar_in[:], psum1[:])
                nc.gpsimd.collective_compute(
                    "ReduceScatter", OP.add, replica_groups=rg,
                    ins=[ar_in.opt()], outs=[ar_out.opt()])
                ar_ins.append(ar_in)
                ar_outs.append(ar_out)

            suma = sm.tile([16, 32], dt.float32)
            sumb = sm.tile([16, 32], dt.float32)
            nc.scalar.dma_start(suma[:], ar_outs[0][:])
            nc.gpsimd.dma_start(sumb[:], ar_outs[1][:])
            sum1 = sm.tile([16, 32], dt.float32)
            nc.vector.tensor_add(sum1[:], suma[:], sumb[:])

            # ---------- phase 4: softmaxes on own batch [16, 32] -----------
            # (values are ~N(0, sigma<4): exp is safe without max-shift)
            def v216(ap):
                return ap.rearrange("p (two s2) -> p two s2", two=2)
            e1 = sm.tile([16, 32], dt.float32)
            nc.scalar.activation(e1[:], sum1[:], ACT.Exp)
            smsum = sm.tile([16, 2], dt.float32)
            nc.vector.tensor_reduce(out=smsum[:, :, None], in_=v216(e1[:]),
                                    axis=AX.X, op=OP.add)
            rsm = sm.tile([16, 2], dt.float32)
            nc.vector.reciprocal(rsm[:], smsum[:])
            nc.vector.tensor_tensor(
                out=v216(e1[:]), in0=v216(e1[:]),
                in1=rsm[:, :, None].to_broadcast([16, 2, 16]), op=OP.mult)

            mult2 = sm.tile([16, 32], dt.float32)
            nc.vector.tensor_tensor(out=mult2[:], in0=e1[:], in1=ssb_ps[:],
                                    op=OP.mult)
            cn = sm.tile([1, 512], dt.float32)
            nc.sync.dma_start(cn[:].rearrange("p (a bb) -> p a bb", a=16),
                              mult2[:])

            # softmax over 512 (|mult2| < ~4: exp safe without max-shift)
            e5 = sm.tile([1, 512], dt.float32)
            nc.scalar.activation(e5[:], cn[:], ACT.Exp)
            s5 = sm.tile([1, 1], dt.float32)
            nc.vector.tensor_reduce(out=s5[:], in_=e5[:], axis=AX.X,
                                    op=OP.add)
            r5 = sm.tile([1, 1], dt.float32)
            nc.vector.reciprocal(r5[:], s5[:])
            cand = sm.tile([1, 512], dt.float32)
            nc.vector.tensor_tensor(out=cand[:], in0=e5[:],
                                    in1=r5[:].to_broadcast([1, 512]),
                                    op=OP.mult)

            # ---------- phase 5: dup sums + exact denominator + scatter ----
            ones128 = sm.tile([1, 128], dt.float32)
            nc.vector.memset(ones128[:], 1.0)
            cb_ps = mps.tile([128, 512], dt.float32, tag="mm")
            nc.tensor.matmul(cb_ps[:], ones128[:], cand[:], start=True,
                             stop=True)
            nc.vector.tensor_tensor(
                out=eqm[:], in0=eqm[:],
                in1=cb_ps[:, None, :].to_broadcast([128, 4, 512]),
                op=OP.mult)
            dup = sm.tile([128, 4], dt.float32)
            nc.vector.tensor_reduce(out=dup[:, :, None], in_=eqm[:],
                                    axis=AX.X, op=OP.add)

            # exp(dup) (dup <= 1, no stabilization needed)
            ex = sm.tile([128, 4], dt.float32)
            nc.scalar.activation(ex[:], dup[:], ACT.Exp)
            exm = sm.tile([128, 4], dt.float32)
            nc.vector.tensor_tensor(out=exm[:], in0=ex[:],
                                    in1=hc[:, HC_FIRST:HC_FIRST + 4],
                                    op=OP.mult)
            # cross-partition total, replicated on every partition
            exall = sm.tile([128, 4], dt.float32)
            nc.gpsimd.partition_all_reduce(exall[:], exm[:], channels=128,
                                           reduce_op=bass_isa.ReduceOp.add)
            sedB = sm.tile([128, 1], dt.float32)
            nc.vector.tensor_reduce(out=sedB[:], in_=exall[:], axis=AX.X,
                                    op=OP.add)
            nc.vector.tensor_add(sedB[:], sedB[:],
                                 hc[:, HC_NBASE:HC_NBASE + 1])
            rdenB = sm.tile([128, 1], dt.float32)
            nc.vector.reciprocal(rdenB[:], sedB[:])

            # base fill of the whole output (copy on Act; DVE is busy)
            fill = sm.tile([128, FILL_W], dt.float32)
            nc.scalar.copy(fill[:], rdenB[:].to_broadcast([128, FILL_W]))
            fill_insts = [
                nc.sync.dma_start(out[:, q * FILL_W:(q + 1) * FILL_W],
                                  fill[:])
                for q in range(4)
            ]

            # delta = firstocc * (exp(dup) - 1) / denom; src = delta x onehot
            exm1 = sm.tile([128, 4], dt.float32)
            nc.vector.scalar_tensor_tensor(
                out=exm1[:], in0=ex[:], scalar=-1.0,
                in1=hc[:, HC_FIRST:HC_FIRST + 4],
                op0=OP.add, op1=OP.mult)
            nc.vector.tensor_tensor(out=exm1[:], in0=exm1[:],
                                    in1=rdenB[:].to_broadcast([128, 4]),
                                    op=OP.mult)
            src = sm.tile([128, 4, SROWS], dt.float32)
            nc.vector.tensor_tensor(
                out=src[:],
                in0=exm1[:, :, None].to_broadcast([128, 4, SROWS]),
                in1=hc[:, HC_OH64:HC_OH64 + 256].rearrange(
                    "p (a c) -> p a c", a=4),
                op=OP.mult)

            scat = nc.gpsimd.dma_scatter_add(
                out_ap=out[:].rearrange("p f -> (p f)").rearrange(
                    "(r c) -> r c", c=SROWS),
                in_ap=src[:],
                idxs_ap=sidx_sb[:],
                num_idxs=512, num_idxs_reg=512, elem_size=SROWS,
                single_packet=False)
            for fi in fill_insts:
                tile.add_dep_helper(scat.ins, fi.ins, sync=True,
                                    reason="base fill before scatter-add")

    nc.compile()
    return nc


def _host_prep(span_embs, triplet_ids_tr, offsets_tr, attention_tr, qid_inds,
               emb_weight, span_W, span_b):
    span_embs = np.asarray(span_embs, dtype=np.float32)
    ids = np.asarray(triplet_ids_tr).astype(np.int64)
    offs = np.asarray(offsets_tr).astype(np.int64)
    att = np.asarray(attention_tr, dtype=np.float32)
    qid = np.asarray(qid_inds).astype(np.int64)
    emb_weight = np.asarray(emb_weight, dtype=np.float32)
    span_W = np.asarray(span_W, dtype=np.float32)
    span_b = np.asarray(span_b, dtype=np.float32)

    # bag id per element (general sorted offsets, offs[b,0] == 0)
    pos = np.arange(L)
    seg = np.empty((B, L), dtype=np.int64)
    for b in range(B):
        seg[b] = np.searchsorted(offs[b], pos, side='right') - 1

    bcol = (np.arange(B)[:, None] * 16 + (seg % 16))        # p = b*16 + m%16
    # device j2 axis: j2 = (c//16)*16 + s, groups contiguous for softmax
    j2 = ((seg // 16) % 2) * 16 + seg // 32
    k_of = ids // TS
    lid = (ids % TS).astype(np.int64)
    halfsel = (lid >= HALF).astype(np.int64)
    lidx = lid - HALF * halfsel

    # rank within (core, half, partition) group, in stable order
    key = ((k_of * 2 + halfsel) * 128 + bcol).ravel()
    order = np.argsort(key, kind='stable')
    sk = key[order]
    starts = np.r_[0, np.flatnonzero(sk[1:] != sk[:-1]) + 1]
    group_id = np.cumsum(np.r_[0, (sk[1:] != sk[:-1]).astype(np.int64)])
    rank_sorted = np.arange(sk.size) - starts[group_id]
    rank = np.empty(sk.size, dtype=np.int64)
    rank[order] = rank_sorted

    max_rank = int(rank.max())
    chi = max(8, (max_rank + 8) // 8)     # slots per partition per chunk
    nslot = NCH * chi
    gw = chi * 128 // 16

    kf = k_of.ravel()
    pf = bcol.ravel()
    j2f = j2.ravel()
    lf = lidx.ravel()
    af = att.ravel().astype(np.float32)
    cf = halfsel.ravel() * (NCH // 2) + rank // chi   # chunk
    ilocf = rank % chi

    # spans layouts
    spans_all = np.ascontiguousarray(span_embs.reshape(128, E))
    spansT = np.ascontiguousarray(spans_all.T)               # [768, 128]
    spansT6 = np.ascontiguousarray(
        spansT.reshape(6, 128, 128).transpose(1, 0, 2).reshape(128, 768)
    ).astype(ml_dtypes.bfloat16)
    WT = np.ascontiguousarray(emb_weight.T)                  # [768, 100000]

    r = np.arange(128)
    hostb = (r[:, None] // 16 == r[None, :] // 16).astype(np.float32)
    hostm = (r[:, None] % 16 == np.arange(32)[None, :] % 16).astype(
        np.float32)
    ident64 = (np.arange(64)[None, :] == (r % 64)[:, None]).astype(
        np.float32)

    x = np.arange(512)
    j2d = x % 32
    mx_map = x // 32 + 16 * (2 * (j2d % 16) + j2d // 16)   # position x -> bag

    in_maps = []
    for k in range(N_CORES):
        sel = kf == k
        p_k, j2_k = pf[sel], j2f[sel]
        l_k, a_k = lf[sel], af[sel]
        c_k, il_k = cf[sel], ilocf[sel]

        # gather idxs: quarter-row id = lid*2 + p//64
        gidx_flat = np.zeros((NCH, chi * 128), dtype=np.int16)
        gidx_flat[c_k, il_k * 128 + p_k] = (l_k * 2 + p_k // 64).astype(
            np.int16)
        gidx = np.zeros((128, NCH * gw), dtype=np.int16)
        for c in range(NCH):
            wrapped = gidx_flat[c].reshape(gw, 16).T       # [16, gw]
            gidx[:, c * gw:(c + 1) * gw] = np.tile(wrapped, (8, 1))

        satt = np.zeros((128, NCH, 32, chi), dtype=np.float32)
        satt[p_k, c_k, j2_k, il_k] = a_k

        own = k
        hostblk = np.zeros((128, 16), dtype=np.float32)
        hostblk[own * 16:(own + 1) * 16, :] = 1.0

        qx = qid[own][mx_map]                               # [512]
        valid = qx < NE
        firstocc = np.zeros(512, dtype=np.float32)
        _, first_idx = np.unique(qx, return_index=True)
        for fi in first_idx:
            if valid[fi]:
                firstocc[fi] = 1.0
        n_distinct = int(np.unique(qx[valid]).size)
        nbase = float(NE - n_distinct)

        onehot64 = np.zeros((512, SROWS), dtype=np.float32)
        onehot64[np.arange(512), qx % SROWS] = 1.0

        # scatter idx list position i = q*128 + p  ->  slot (p, q)
        i = np.arange(512)
        pos_i = (i % 128) * 4 + (i // 128)
        sidx_flat = (qx[pos_i] // SROWS).astype(np.int16)
        sidx = np.tile(sidx_flat.reshape(32, 16).T, (8, 1))  # [128, 32]

        hconst = np.zeros((128, HC_W), dtype=np.float32)
        hconst[:, HC_SPANS:HC_SPANS + E] = spans_all
        hconst[:, HC_SPANW:HC_SPANW + E] = span_W[:, 0][None, :]
        hconst[:, HC_HOSTB:HC_HOSTB + 128] = hostb
        hconst[:, HC_HOSTM:HC_HOSTM + 32] = hostm
        hconst[:, HC_BLK:HC_BLK + 16] = hostblk
        hconst[:, HC_ID64:HC_ID64 + 64] = ident64
        hconst[:, HC_QIDP:HC_QIDP + 4] = qx.reshape(128, 4)
        hconst[:, HC_QIDF:HC_QIDF + 512] = qx[None, :]
        hconst[:, HC_FIRST:HC_FIRST + 4] = firstocc.reshape(128, 4)
        hconst[:, HC_NBASE] = nbase
        hconst[:, HC_OH64:HC_OH64 + 256] = onehot64.reshape(128, 256)
        hconst[:, HC_SPANB] = float(span_b[0])

        in_maps.append(dict(
            wt=np.ascontiguousarray(
                WT[:, k * TS:(k + 1) * TS]).astype(ml_dtypes.bfloat16),
            spansT6=spansT6,
            hconst=hconst,
            sidx=sidx,
            gidx=gidx,
            satt=np.ascontiguousarray(
                satt.reshape(128, -1)).astype(ml_dtypes.bfloat16),
        ))
    return in_maps, nslot


def kernel_run(inputs, trace=False):
    in_maps, nslot = _host_prep(**inputs)
    if nslot not in _cache:
        _cache[nslot] = _build(nslot)
    nc = _cache[nslot]
    res = run_bass_kernel_spmd(nc, in_maps, core_ids=list(range(N_CORES)),
                               trace=trace)
    out = np.stack([r["out"].reshape(-1)[:NE] for r in res.results])
    return out[:, :, None].astype(np.float32), res


def kernel(**inputs):
    out, _ = kernel_run(inputs)
    return out
